# revision 1
# baseline (speedup 1.0000x reference)
"""AdaptiveGaussianTrendCausal Trainium2 kernel (8-core data parallel).

Strategy (per core, B_loc=4, T=4096, C=64; elements laid out as
[128 position partitions x 256 (b,c) free] chunk tiles):

Phase A (ScalarE table set: natural_log_exp):
  - causal depthwise convs as banded-Toeplitz matmuls (cur+prev chunk
    accumulated in PSUM). 5 gaussian filters in f32r, box sums (win=16)
    for running stats in exact f32 (variance cancellation safety).
  - running mean/var on VectorE, logv = Ln(var+1e-6),
    rsqrt via exp(-0.5*logv)  (avoids sqrt table set entirely),
    z = (x-mean)*rsq.  Y_k saved to SBUF in bf16 via ScalarE copies.

Phase B (table set: gelu):
  - DMA-rearrange z/logv into blockdiag moving layout [8, .] (4 elems/col)
  - L1 blockdiag matmul -> PSUM h [128 = 4 elems x 32 hidden, .]
  - exact Gelu with per-partition bias b1 -> SBUF
  - L2 blockdiag matmul -> logits PSUM [20 = 4 elems x 5 k, .]
  - DMA logits to DRAM scratch in a k-plane friendly layout

Phase C/D (table set: natural_log_exp):
  - reload logits as full-partition k-plane chunks
  - e_k = Exp(logits/0.7 + b2_k/0.7)   (bias folded into activation)
    no max-subtract needed: max |logit|/0.7 ~ 42 << 88 (fp32 exp limit)
  - den = sum_k e_k, rden = exp(-ln(den)), num = sum_k Y_k*e_k (bf16 DVE)
  - trend = num * rden -> DMA out
"""

import os
import sys
import numpy as np

for _p in ("/opt/trn_rl_repo",):
    if _p not in sys.path and os.path.isdir(_p):
        sys.path.insert(0, _p)

from concourse import bass, mybir
from concourse import bacc

# Constrain the activation-table chooser: serve ln/exp only from the combined
# natural_log_exp set (and keep gelu's set pure) so phase-interleaved ACT
# streams don't ping-pong table loads. List order/IDs are preserved.
import concourse.hw_specs as _hw_specs
_orig_get_tables = _hw_specs.get_activation_tables
def _pinned_tables(module_arch):
    tabs = _orig_get_tables(module_arch)
    out = {}
    for name, funcs in tabs.items():
        f = set(funcs)
        if name not in ("natural_log_exp_and_others",):
            f.discard(mybir.ActivationFunctionType.Ln)
            f.discard(mybir.ActivationFunctionType.Exp)
        out[name] = f
    return out
_hw_specs.get_activation_tables = _pinned_tables
bacc.get_activation_tables = _pinned_tables
from concourse.tile import TileContext
from concourse.bass_utils import run_bass_kernel_spmd

F32 = mybir.dt.float32
F32R = mybir.dt.float32r
BF16 = mybir.dt.bfloat16
F16 = mybir.dt.float16
AF = mybir.ActivationFunctionType

B, T, C, H, NK = 32, 4096, 64, 32, 5
NCORES = 8
BL = B // NCORES          # 4 batch elems per core
BC = BL * C               # 256 free columns per chunk
PCH = 128                 # positions per chunk
NCH = T // PCH            # 32 chunks
SIGMAS = (2.5, 4.0, 6.0, 9.0, 14.0)
WIN = 16
TEMP = 0.7

LAST_EXEC_NS = None
LAST_RESULTS = None


def _gauss_kernel_np(sigma):
    # matches reference._gauss_kernel in float32 arithmetic
    R = min(max(1, int(4.0 * sigma + 0.5)), max(1, T - 1))
    n = np.arange(0, R + 1, dtype=np.float32)
    k = np.exp(-0.5 * (n / np.float32(max(float(sigma), 1e-6))) ** 2).astype(np.float32)
    return (k / (k.sum() + np.float32(1e-12))).astype(np.float32)


def _band_mats(k):
    """Toeplitz pair (Acur, Aprev) with out = Acur.T@x_cur + Aprev.T@x_prev.

    Acur[pi, po] = k[po - pi]        for 0 <= po-pi <= R
    Aprev[pi, po] = k[po + 128 - pi] for 1 <= po+128-pi <= R
    (chunk0 uses x_prev = x[0] replicated -> exactly the edge padding)
    """
    R = len(k) - 1
    cur = np.zeros((PCH, PCH), np.float32)
    prv = np.zeros((PCH, PCH), np.float32)
    for pi in range(PCH):
        for po in range(PCH):
            d = po - pi
            if 0 <= d <= R:
                cur[pi, po] = k[d]
            d2 = po + PCH - pi
            if 1 <= d2 <= R:
                prv[pi, po] = k[d2]
    return cur, prv


def _build_consts(W1, b1, W2):
    convm = np.zeros((12, PCH, PCH), np.float32)
    for f, s in enumerate(SIGMAS):
        cur, prv = _band_mats(_gauss_kernel_np(s)[::-1].copy())
        convm[2 * f] = cur
        convm[2 * f + 1] = prv
    bcur, bprv = _band_mats(np.ones(WIN, np.float32))
    convm[10] = bcur
    convm[11] = bprv

    w1blk = np.zeros((8, 128), np.float32)
    b1vec = np.zeros((128, 1), np.float32)
    w2blk = np.zeros((128, 6 * 120), np.float32)
    for m in range(4):
        for j in range(H):
            w1blk[2 * m, 32 * m + j] = W1[j, 0] * 4.0
            w1blk[2 * m + 1, 32 * m + j] = W1[j, 1]
            b1vec[32 * m + j, 0] = b1[j]
            for a in range(6):
                for k in range(NK):
                    w2blk[32 * m + j, 120 * a + 20 * a + 5 * m + k] = W2[k, j] / 16.0

    eff = np.minimum(np.arange(1, PCH + 1, dtype=np.float32), np.float32(WIN))
    r0 = (np.float32(1.0) / (eff + np.float32(1e-12))).astype(np.float32)
    r0b = np.broadcast_to(r0[:, None], (PCH, BC)).copy()
    return convm, w1blk, b1vec, w2blk, r0b


def _build_graph(b2, dbg=False, phases=3):
    nc = bacc.Bacc()
    x_d = nc.declare_dram_parameter("x", [BL, T, C], F32, isOutput=False)
    convr_d = nc.declare_dram_parameter("convmr", [PCH, 12, PCH], F32R, isOutput=False)
    w1r_d = nc.declare_dram_parameter("w1blkr", [8, 128], F16, isOutput=False)
    w2r_d = nc.declare_dram_parameter("w2blkr", [128, 6 * 120], F32R, isOutput=False)
    x0r_d = nc.declare_dram_parameter("x0repr", [PCH, BC], F32R, isOutput=False)
    b1_d = nc.declare_dram_parameter("b1vec", [128, 1], F32, isOutput=False)
    r0_d = nc.declare_dram_parameter("r0b", [PCH, BC], F32, isOutput=False)
    x0_d = nc.declare_dram_parameter("x0rep", [PCH, BC], F32, isOutput=False)
    bias_d = nc.declare_dram_parameter("biasv", [PCH, 7], F32, isOutput=False)
    out_d = nc.declare_dram_parameter("out", [BL, T, C], F32, isOutput=True)
    # logits scratch [k, p4, m, bc]
    TP = 4224  # T padded to a multiple of 384 (48-pos units x 128-pos chunks)
    if dbg:
        lg_d = nc.declare_dram_parameter("lgdbg", [TP // 4, 4, NK, BC], F16, isOutput=True)
        zl_d = nc.declare_dram_parameter("zldbg", [TP // 4, 4, 2, BC], F16, isOutput=True)
    else:
        lg_d = nc.dram_tensor("lgscratch", [TP // 4, 4, NK, BC], F16)
        zl_d = nc.dram_tensor("zlscratch", [TP // 4, 4, 2, BC], F16)

    inv_t = float(16.0 / TEMP)

    with TileContext(nc) as tc:
        with tc.tile_pool(name="const", bufs=1) as cpool:

            b1_sb = cpool.tile([128, 1], F32)
            nc.sync.dma_start(out=b1_sb, in_=b1_d[:])
            r0_sb = cpool.tile([PCH, BC], F32)
            nc.sync.dma_start(out=r0_sb, in_=r0_d[:])
            x0_sb = cpool.tile([PCH, BC], F32)
            nc.sync.dma_start(out=x0_sb, in_=x0_d[:])
            x02_sb = cpool.tile([PCH, BC], F32R)
            bias_sb = cpool.tile([PCH, 7], F32)
            nc.sync.dma_start(out=bias_sb, in_=bias_d[:])
            conv_sbr = cpool.tile([PCH, 12 * PCH], F32R)
            nc.sync.dma_start(out=conv_sbr, in_=convr_d.rearrange("p i q -> p (i q)"))
            w1_sbr = cpool.tile([8, 128], F16)
            nc.sync.dma_start(out=w1_sbr, in_=w1r_d[:])
            w2_sbr = cpool.tile([128, 6 * 120], F32R)
            nc.sync.dma_start(out=w2_sbr, in_=w2r_d[:])
            x0_sbr = cpool.tile([PCH, BC], F32R)
            nc.sync.dma_start(out=x0_sbr, in_=x0r_d[:])


            with tc.tile_pool(name="feat", bufs=1) as fpool:
                y_all = fpool.tile([PCH, NCH * NK * BC], BF16)  # free = (ch, k, bc)

                # ---------------- Phase A ----------------
                with tc.tile_pool(name="xp", bufs=6) as xpool, \
                     tc.tile_pool(name="x2p", bufs=4) as x2pool, \
                     tc.tile_pool(name="cps", bufs=2, space="PSUM") as cpsum, \
                     tc.tile_pool(name="stat", bufs=4) as spool:
                    nc.vector.tensor_mul(x02_sb, x0_sb, x0_sb)
                    xprev_r, x2prev = x0_sbr, x02_sb
                    for ch in range(NCH):
                        xt = xpool.tile([PCH, BC], F32, tag="x")
                        nc.sync.dma_start(
                            out=xt.rearrange("p (b c) -> p b c", b=BL),
                            in_=x_d[:, ch * PCH:(ch + 1) * PCH, :]
                                .rearrange("b p c -> p b c"))
                        xtr = xpool.tile([PCH, BC], F32R, tag="xr")
                        nc.vector.tensor_copy(xtr, xt)
                        x2 = x2pool.tile([PCH, BC], F32R, tag="x2")
                        nc.scalar.activation(out=x2, in_=xt, func=AF.Square)
                        psg = cpsum.tile([PCH, 5 * BC], F32, tag="cvg")
                        pst = cpsum.tile([PCH, 2 * BC], F32, tag="cvs")
                        for f in range(5):
                            nc.tensor.matmul(
                                psg[:, f * BC:(f + 1) * BC],
                                lhsT=conv_sbr[:, (2 * f) * PCH:(2 * f + 1) * PCH],
                                rhs=xtr, start=True, stop=False)
                            nc.tensor.matmul(
                                psg[:, f * BC:(f + 1) * BC],
                                lhsT=conv_sbr[:, (2 * f + 1) * PCH:(2 * f + 2) * PCH],
                                rhs=xprev_r, start=False, stop=True)
                        nc.tensor.matmul(pst[:, 0:BC],
                                         lhsT=conv_sbr[:, 10 * PCH:11 * PCH],
                                         rhs=xtr, start=True, stop=False)
                        nc.tensor.matmul(pst[:, 0:BC],
                                         lhsT=conv_sbr[:, 11 * PCH:12 * PCH],
                                         rhs=xprev_r, start=False, stop=True)
                        nc.tensor.matmul(pst[:, BC:2 * BC],
                                         lhsT=conv_sbr[:, 10 * PCH:11 * PCH],
                                         rhs=x2, start=True, stop=False)
                        nc.tensor.matmul(pst[:, BC:2 * BC],
                                         lhsT=conv_sbr[:, 11 * PCH:12 * PCH],
                                         rhs=x2prev, start=False, stop=True)
                        nc.scalar.activation(
                            out=y_all[:, ch * NK * BC:(ch + 1) * NK * BC],
                            in_=psg, func=AF.Copy)
                        mean = spool.tile([PCH, BC], F32, tag="mean")
                        mean2 = spool.tile([PCH, BC], F32, tag="mean2")
                        if ch == 0:
                            nc.vector.tensor_mul(mean, pst[:, 0:BC], r0_sb)
                            nc.vector.tensor_mul(mean2, pst[:, BC:2 * BC], r0_sb)
                        else:
                            nc.vector.tensor_scalar_mul(mean, pst[:, 0:BC], 1.0 / WIN)
                            nc.vector.tensor_scalar_mul(mean2, pst[:, BC:2 * BC], 1.0 / WIN)
                        msq = spool.tile([PCH, BC], F32, tag="msq")
                        nc.scalar.activation(out=msq, in_=mean, func=AF.Square)
                        var = spool.tile([PCH, BC], F32, tag="var")
                        nc.vector.tensor_sub(var, mean2, msq)
                        nc.vector.tensor_scalar_max(var, var, 0.0)
                        lv = spool.tile([PCH, BC], F16, tag="lv")
                        nc.scalar.activation(
                            out=lv, in_=var, func=AF.Ln, bias=bias_sb[:, 0:1])
                        rsq = spool.tile([PCH, BC], F32, tag="rsq")
                        nc.scalar.activation(out=rsq, in_=lv, func=AF.Exp, scale=-0.5,
                                             bias=bias_sb[:, 6:7])
                        xm = spool.tile([PCH, BC], F32, tag="xm")
                        nc.vector.tensor_sub(xm, xt, mean)
                        zt = spool.tile([PCH, BC], F16, tag="zt")
                        nc.vector.tensor_mul(zt, xm, rsq)
                        nc.sync.dma_start(
                            out=zl_d[ch * 32:(ch + 1) * 32, :, 0, :]
                                .rearrange("p4 m bc -> (p4 m) bc"),
                            in_=zt)
                        nc.sync.dma_start(
                            out=zl_d[ch * 32:(ch + 1) * 32, :, 1, :]
                                .rearrange("p4 m bc -> (p4 m) bc"),
                            in_=lv)
                        if ch == NCH - 1:
                            nc.sync.dma_start(
                                out=zl_d[1024:1056, :, 0, :]
                                    .rearrange("p4 m bc -> (p4 m) bc"),
                                in_=zt)
                            nc.sync.dma_start(
                                out=zl_d[1024:1056, :, 1, :]
                                    .rearrange("p4 m bc -> (p4 m) bc"),
                                in_=lv)
                        xprev_r, x2prev = xtr, x2

                # ---------------- Phase B ----------------
                if phases < 2:
                    phases_skip = True
                else:
                    phases_skip = False
                with tc.tile_pool(name="x1p", bufs=3) as x1pool, \
                     tc.tile_pool(name="hps", bufs=2, space="PSUM") as hpsum, \
                     tc.tile_pool(name="hsb", bufs=3) as hpool, \
                     tc.tile_pool(name="l2ps", bufs=2, space="PSUM") as l2psum, \
                     tc.tile_pool(name="lcp", bufs=4) as lcpool:
                    for up in range(88 if not phases_skip else 0):  # unit pairs: 48 positions each
                        x1 = x1pool.tile([8, 12 * BC], F16, tag="x1")
                        nc.sync.dma_start(
                            out=x1.rearrange("mf (p4 bc) -> mf p4 bc", p4=12),
                            in_=zl_d[12 * up:12 * up + 12]
                                .rearrange("p4 m f bc -> (m f) p4 bc"))
                        l2 = l2psum.tile([120, 512], F32, tag="l2")
                        for ul in range(2):
                            hp = hpsum.tile([128, 6 * BC], F32, tag="h")
                            for a in range(6):
                                blk = ul * 6 + a
                                nc.tensor.matmul(
                                    hp[:, a * BC:(a + 1) * BC],
                                    lhsT=w1_sbr,
                                    rhs=x1[:, blk * BC:(blk + 1) * BC],
                                    start=True, stop=True)
                            hs = hpool.tile([128, 6 * BC], F32R, tag="hs")
                            nc.scalar.activation(out=hs, in_=hp, func=AF.Gelu, bias=b1_sb)
                            for a in range(6):
                                nc.tensor.matmul(
                                    l2[:, ul * BC:(ul + 1) * BC],
                                    lhsT=w2_sbr[:, a * 120:(a + 1) * 120],
                                    rhs=hs[:, a * BC:(a + 1) * BC],
                                    start=(a == 0), stop=(a == 5))
                        lcp = lcpool.tile([120, 512], F16, tag="lc")
                        nc.vector.tensor_copy(lcp, l2)
                        for ul in range(2):
                            p4a = 12 * up + 6 * ul
                            nc.sync.dma_start(
                                out=lg_d[p4a:p4a + 6, :, :, :]
                                    .rearrange("p4 m k bc -> (p4 m k) bc"),
                                in_=lcp[:, ul * BC:(ul + 1) * BC])

                # ---------------- Phase C/D ----------------
                tc.no_sync_barrier()
                NSC = 8 if phases >= 3 else 0
                SCW = 4 * BC  # 1024
                with tc.tile_pool(name="lgp", bufs=2) as lpool, \
                     tc.tile_pool(name="ep", bufs=2) as epool, \
                     tc.tile_pool(name="dp", bufs=2) as dpool, \
                     tc.tile_pool(name="op", bufs=2) as opool:
                    for sc in range(NSC):
                        et = epool.tile([PCH, NK * SCW], BF16, tag="e")
                        lgt = lpool.tile([PCH, NK * SCW], F16, tag="lg")
                        for k in range(NK):
                            nc.scalar.dma_start(
                                out=lgt[:, k * SCW:(k + 1) * SCW]
                                    .rearrange("p (q bc) -> p q bc", q=4),
                                in_=lg_d[sc * 128:(sc + 1) * 128, :, k, :]
                                    .rearrange("(q p4) m bc -> (p4 m) q bc", q=4))
                        if sc == 0:
                            # only t<16 can overflow exp: per-element max-subtract here
                            mx = lpool.tile([PCH, SCW], F16, tag="mx")
                            nc.vector.tensor_max(mx, lgt[:, 0:SCW], lgt[:, SCW:2 * SCW])
                            for k in (2, 3, 4):
                                nc.vector.tensor_max(mx, mx, lgt[:, k * SCW:(k + 1) * SCW])
                            for k in range(NK):
                                dk = lpool.tile([PCH, SCW], F16, tag="dk")
                                nc.vector.tensor_sub(dk, lgt[:, k * SCW:(k + 1) * SCW], mx)
                                nc.scalar.activation(
                                    out=et[:, k * SCW:(k + 1) * SCW], in_=dk,
                                    func=AF.Exp, scale=inv_t, bias=bias_sb[:, 1 + k:2 + k])
                        else:
                            for k in range(NK):
                                nc.scalar.activation(
                                    out=et[:, k * SCW:(k + 1) * SCW],
                                    in_=lgt[:, k * SCW:(k + 1) * SCW],
                                    func=AF.Exp, scale=inv_t, bias=bias_sb[:, 1 + k:2 + k])
                        den = dpool.tile([PCH, SCW], BF16, tag="den")
                        nc.vector.tensor_add(den, et[:, 0:SCW], et[:, SCW:2 * SCW])
                        for k in (2, 3, 4):
                            nc.vector.tensor_add(den, den, et[:, k * SCW:(k + 1) * SCW])
                        ld = dpool.tile([PCH, SCW], F32, tag="ld")
                        nc.scalar.activation(out=ld, in_=den, func=AF.Ln)
                        rd = dpool.tile([PCH, SCW], BF16, tag="rd")
                        nc.scalar.activation(out=rd, in_=ld, func=AF.Exp, scale=-1.0)
                        yv = y_all.rearrange("p (ch k bc) -> p ch k bc", ch=NCH, k=NK)
                        ysl = lambda k: yv[:, 4 * sc:4 * sc + 4, k, :]
                        num = dpool.tile([PCH, SCW], BF16, tag="num")
                        tmp = dpool.tile([PCH, SCW], BF16, tag="tmp")
                        nc.vector.tensor_mul(num, et[:, 0:SCW], ysl(0))
                        for k in range(1, NK):
                            nc.vector.tensor_mul(tmp, et[:, k * SCW:(k + 1) * SCW], ysl(k))
                            nc.vector.tensor_add(num, num, tmp)
                        ot = opool.tile([PCH, SCW], F32, tag="ot")
                        nc.vector.tensor_mul(ot, num, rd)
                        for q in range(4):
                            nc.sync.dma_start(
                                out=out_d[:, sc * 512 + q * 128:sc * 512 + (q + 1) * 128, :]
                                    .rearrange("b p c -> p b c"),
                                in_=ot[:, q * BC:(q + 1) * BC]
                                    .rearrange("p (b c) -> p b c", b=BL))
    nc.compile()
    return nc


def kernel(x, W1, b1, W2, b2, _trace=False, _dbg=False):
    global LAST_EXEC_NS, LAST_RESULTS
    x = np.ascontiguousarray(np.asarray(x, np.float32))
    W1 = np.asarray(W1, np.float32)
    b1 = np.asarray(b1, np.float32)
    W2 = np.asarray(W2, np.float32)
    b2 = np.asarray(b2, np.float32)

    convm, w1blk, b1vec, w2blk, r0b = _build_consts(W1, b1, W2)
    convm = np.ascontiguousarray(convm.transpose(1, 0, 2))  # [pi, i, po]
    bvals = np.concatenate([[np.float32(1e-6)],
                            (b2 / np.float32(TEMP)).astype(np.float32),
                            [np.float32(np.log(0.25))]])
    biasv = np.ascontiguousarray(np.broadcast_to(bvals[None, :], (PCH, 7)).astype(np.float32))
    nc = _build_graph(b2, dbg=_dbg)

    in_maps = []
    for i in range(NCORES):
        xs = x[i * BL:(i + 1) * BL]
        x0rep = np.broadcast_to(
            xs[:, 0, :].reshape(1, BC), (PCH, BC)).copy().astype(np.float32)
        in_maps.append({
            "x": np.ascontiguousarray(xs),
            "convmr": convm,
            "w1blkr": w1blk.astype(np.float16), "b1vec": b1vec,
            "w2blkr": w2blk, "r0b": r0b,
            "x0rep": x0rep, "x0repr": x0rep,
            "biasv": biasv,
        })
    res = run_bass_kernel_spmd(nc, in_maps, list(range(NCORES)), trace=_trace)
    LAST_EXEC_NS = res.exec_time_ns
    LAST_RESULTS = res
    out = np.concatenate([res.results[i]["out"] for i in range(NCORES)], axis=0)
    return out.astype(np.float32)


if __name__ == "__main__":
    rng = np.random.default_rng(0)
    xs = rng.standard_normal((B, T, C)).astype(np.float32)
    W1 = rng.standard_normal((H, 2)).astype(np.float32) * 0.5
    b1 = rng.standard_normal((H,)).astype(np.float32) * 0.1
    W2 = rng.standard_normal((NK, H)).astype(np.float32) * 0.3
    b2 = rng.standard_normal((NK,)).astype(np.float32) * 0.1
    o = kernel(xs, W1, b1, W2, b2)
    print("ran", o.shape, o.dtype)



# revision 11
# speedup vs baseline: 1.0536x; 1.0536x over previous
"""AdaptiveGaussianTrendCausal Trainium2 kernel (8-core data parallel).

Strategy (per core, B_loc=4, T=4096, C=64; elements laid out as
[128 position partitions x 256 (b,c) free] chunk tiles):

Phase A (ScalarE table set: natural_log_exp):
  - causal depthwise convs as banded-Toeplitz matmuls (cur+prev chunk
    accumulated in PSUM). 5 gaussian filters in f32r, box sums (win=16)
    for running stats in exact f32 (variance cancellation safety).
  - running mean/var on VectorE, logv = Ln(var+1e-6),
    rsqrt via exp(-0.5*logv)  (avoids sqrt table set entirely),
    z = (x-mean)*rsq.  Y_k saved to SBUF in bf16 via ScalarE copies.

Phase B (table set: gelu):
  - DMA-rearrange z/logv into blockdiag moving layout [8, .] (4 elems/col)
  - L1 blockdiag matmul -> PSUM h [128 = 4 elems x 32 hidden, .]
  - exact Gelu with per-partition bias b1 -> SBUF
  - L2 blockdiag matmul -> logits PSUM [20 = 4 elems x 5 k, .]
  - DMA logits to DRAM scratch in a k-plane friendly layout

Phase C/D (table set: natural_log_exp):
  - reload logits as full-partition k-plane chunks
  - e_k = Exp(logits/0.7 + b2_k/0.7)   (bias folded into activation)
    no max-subtract needed: max |logit|/0.7 ~ 42 << 88 (fp32 exp limit)
  - den = sum_k e_k, rden = exp(-ln(den)), num = sum_k Y_k*e_k (bf16 DVE)
  - trend = num * rden -> DMA out
"""

import os
import sys
import numpy as np

for _p in ("/opt/trn_rl_repo",):
    if _p not in sys.path and os.path.isdir(_p):
        sys.path.insert(0, _p)

from concourse import bass, mybir
from concourse import bacc

# Constrain the activation-table chooser: serve ln/exp only from the combined
# natural_log_exp set (and keep gelu's set pure) so phase-interleaved ACT
# streams don't ping-pong table loads. List order/IDs are preserved.
import concourse.hw_specs as _hw_specs
_orig_get_tables = _hw_specs.get_activation_tables
def _pinned_tables(module_arch):
    tabs = _orig_get_tables(module_arch)
    out = {}
    for name, funcs in tabs.items():
        f = set(funcs)
        if name not in ("natural_log_exp_and_others",):
            f.discard(mybir.ActivationFunctionType.Ln)
            f.discard(mybir.ActivationFunctionType.Exp)
        out[name] = f
    return out
_hw_specs.get_activation_tables = _pinned_tables
bacc.get_activation_tables = _pinned_tables
from concourse.tile import TileContext
from concourse.bass_utils import run_bass_kernel_spmd

F32 = mybir.dt.float32
F32R = mybir.dt.float32r
BF16 = mybir.dt.bfloat16
F16 = mybir.dt.float16
AF = mybir.ActivationFunctionType

B, T, C, H, NK = 32, 4096, 64, 32, 5
NCORES = 8
BL = B // NCORES          # 4 batch elems per core
BC = BL * C               # 256 free columns per chunk
PCH = 128                 # positions per chunk
NCH = T // PCH            # 32 chunks
SIGMAS = (2.5, 4.0, 6.0, 9.0, 14.0)
WIN = 16
TEMP = 0.7

LAST_EXEC_NS = None
LAST_RESULTS = None


def _gauss_kernel_np(sigma):
    # matches reference._gauss_kernel in float32 arithmetic
    R = min(max(1, int(4.0 * sigma + 0.5)), max(1, T - 1))
    n = np.arange(0, R + 1, dtype=np.float32)
    k = np.exp(-0.5 * (n / np.float32(max(float(sigma), 1e-6))) ** 2).astype(np.float32)
    return (k / (k.sum() + np.float32(1e-12))).astype(np.float32)


def _band_mats(k):
    """Toeplitz pair (Acur, Aprev) with out = Acur.T@x_cur + Aprev.T@x_prev.

    Acur[pi, po] = k[po - pi]        for 0 <= po-pi <= R
    Aprev[pi, po] = k[po + 128 - pi] for 1 <= po+128-pi <= R
    (chunk0 uses x_prev = x[0] replicated -> exactly the edge padding)
    """
    R = len(k) - 1
    cur = np.zeros((PCH, PCH), np.float32)
    prv = np.zeros((PCH, PCH), np.float32)
    for pi in range(PCH):
        for po in range(PCH):
            d = po - pi
            if 0 <= d <= R:
                cur[pi, po] = k[d]
            d2 = po + PCH - pi
            if 1 <= d2 <= R:
                prv[pi, po] = k[d2]
    return cur, prv


def _build_consts(W1, b1, W2):
    convm = np.zeros((12, PCH, PCH), np.float32)
    for f, s in enumerate(SIGMAS):
        cur, prv = _band_mats(_gauss_kernel_np(s)[::-1].copy())
        convm[2 * f] = cur
        convm[2 * f + 1] = prv
    bcur, bprv = _band_mats(np.ones(WIN, np.float32))
    convm[10] = bcur
    convm[11] = bprv

    w1blk = np.zeros((8, 128), np.float32)
    b1vec = np.zeros((128, 1), np.float32)
    w2blk = np.zeros((128, 6 * 120), np.float32)
    for m in range(4):
        for j in range(H):
            w1blk[2 * m, 32 * m + j] = W1[j, 0] * 4.0
            w1blk[2 * m + 1, 32 * m + j] = W1[j, 1]
            b1vec[32 * m + j, 0] = b1[j]
            for a in range(6):
                for k in range(NK):
                    w2blk[32 * m + j, 120 * a + 20 * a + 5 * m + k] = W2[k, j] / 16.0

    eff = np.minimum(np.arange(1, PCH + 1, dtype=np.float32), np.float32(WIN))
    r0 = (np.float32(1.0) / (eff + np.float32(1e-12))).astype(np.float32)
    r0b = np.broadcast_to(r0[:, None], (PCH, BC)).copy()
    return convm, w1blk, b1vec, w2blk, r0b


def _build_graph(b2, dbg=False, phases=3):
    nc = bacc.Bacc()
    x_d = nc.declare_dram_parameter("x", [BL, T, C], F32, isOutput=False)
    convr_d = nc.declare_dram_parameter("convmr", [PCH, 12, PCH], F32R, isOutput=False)
    w1r_d = nc.declare_dram_parameter("w1blkr", [8, 128], F16, isOutput=False)
    w2r_d = nc.declare_dram_parameter("w2blkr", [128, 6 * 120], F32R, isOutput=False)
    x0r_d = nc.declare_dram_parameter("x0repr", [PCH, BC], F32R, isOutput=False)
    b1_d = nc.declare_dram_parameter("b1vec", [128, 1], F32, isOutput=False)
    r0_d = nc.declare_dram_parameter("r0b", [PCH, BC], F32, isOutput=False)
    x0_d = nc.declare_dram_parameter("x0rep", [PCH, BC], F32, isOutput=False)
    bias_d = nc.declare_dram_parameter("biasv", [PCH, 7], F32, isOutput=False)
    out_d = nc.declare_dram_parameter("out", [BL, T, C], F32, isOutput=True)
    # logits scratch [k, p4, m, bc]
    TP = 4224  # T padded to a multiple of 384 (48-pos units x 128-pos chunks)
    if dbg:
        lg_d = nc.declare_dram_parameter("lgdbg", [TP // 4, 4, NK, BC], F16, isOutput=True)
        zl_d = nc.declare_dram_parameter("zldbg", [TP // 4, 4, 2, BC], F16, isOutput=True)
    else:
        lg_d = nc.dram_tensor("lgscratch", [TP // 4, 4, NK, BC], F16)
        zl_d = nc.dram_tensor("zlscratch", [TP // 4, 4, 2, BC], F16)

    inv_t = float(16.0 / TEMP)

    with TileContext(nc) as tc:
        with tc.tile_pool(name="const", bufs=1) as cpool:

            b1_sb = cpool.tile([128, 1], F32)
            nc.sync.dma_start(out=b1_sb, in_=b1_d[:])
            r0_sb = cpool.tile([PCH, BC], F32)
            nc.sync.dma_start(out=r0_sb, in_=r0_d[:])
            x0_sb = cpool.tile([PCH, BC], F32)
            nc.sync.dma_start(out=x0_sb, in_=x0_d[:])
            x02_sb = cpool.tile([PCH, BC], F32R)
            bias_sb = cpool.tile([PCH, 7], F32)
            nc.sync.dma_start(out=bias_sb, in_=bias_d[:])
            conv_sbr = cpool.tile([PCH, 12 * PCH], F32R)
            nc.sync.dma_start(out=conv_sbr, in_=convr_d.rearrange("p i q -> p (i q)"))
            w1_sbr = cpool.tile([8, 128], F16)
            nc.sync.dma_start(out=w1_sbr, in_=w1r_d[:])
            w2_sbr = cpool.tile([128, 6 * 120], F32R)
            nc.sync.dma_start(out=w2_sbr, in_=w2r_d[:])
            x0_sbr = cpool.tile([PCH, BC], F32R)
            nc.sync.dma_start(out=x0_sbr, in_=x0r_d[:])


            with tc.tile_pool(name="feat", bufs=1) as fpool:
                y_all = fpool.tile([PCH, NCH * NK * BC], BF16)  # free = (ch, k, bc)

                # ---------------- Phase A ----------------
                with tc.tile_pool(name="xp", bufs=6) as xpool, \
                     tc.tile_pool(name="x2p", bufs=4) as x2pool, \
                     tc.tile_pool(name="cps", bufs=2, space="PSUM") as cpsum, \
                     tc.tile_pool(name="stat", bufs=4) as spool:
                    nc.vector.tensor_mul(x02_sb, x0_sb, x0_sb)
                    xprev_r, x2prev = x0_sbr, x02_sb
                    for ch in range(NCH):
                        xt = xpool.tile([PCH, BC], F32, tag="x")
                        nc.sync.dma_start(
                            out=xt.rearrange("p (b c) -> p b c", b=BL),
                            in_=x_d[:, ch * PCH:(ch + 1) * PCH, :]
                                .rearrange("b p c -> p b c"))
                        xtr = xpool.tile([PCH, BC], F32R, tag="xr")
                        nc.vector.tensor_copy(xtr, xt)
                        x2 = x2pool.tile([PCH, BC], F32R, tag="x2")
                        nc.scalar.activation(out=x2, in_=xt, func=AF.Square)
                        psg = cpsum.tile([PCH, 5 * BC], F32, tag="cvg")
                        pst = cpsum.tile([PCH, 2 * BC], F32, tag="cvs")
                        for f in range(5):
                            nc.tensor.matmul(
                                psg[:, f * BC:(f + 1) * BC],
                                lhsT=conv_sbr[:, (2 * f) * PCH:(2 * f + 1) * PCH],
                                rhs=xtr, start=True, stop=False)
                            nc.tensor.matmul(
                                psg[:, f * BC:(f + 1) * BC],
                                lhsT=conv_sbr[:, (2 * f + 1) * PCH:(2 * f + 2) * PCH],
                                rhs=xprev_r, start=False, stop=True)
                        nc.tensor.matmul(pst[:, 0:BC],
                                         lhsT=conv_sbr[:, 10 * PCH:11 * PCH],
                                         rhs=xtr, start=True, stop=False)
                        nc.tensor.matmul(pst[:, 0:BC],
                                         lhsT=conv_sbr[:, 11 * PCH:12 * PCH],
                                         rhs=xprev_r, start=False, stop=True)
                        nc.tensor.matmul(pst[:, BC:2 * BC],
                                         lhsT=conv_sbr[:, 10 * PCH:11 * PCH],
                                         rhs=x2, start=True, stop=False)
                        nc.tensor.matmul(pst[:, BC:2 * BC],
                                         lhsT=conv_sbr[:, 11 * PCH:12 * PCH],
                                         rhs=x2prev, start=False, stop=True)
                        nc.scalar.activation(
                            out=y_all[:, ch * NK * BC:(ch + 1) * NK * BC],
                            in_=psg, func=AF.Copy)
                        mean = spool.tile([PCH, BC], F32, tag="mean")
                        mean2 = spool.tile([PCH, BC], F32, tag="mean2")
                        if ch == 0:
                            nc.vector.tensor_mul(mean, pst[:, 0:BC], r0_sb)
                            nc.vector.tensor_mul(mean2, pst[:, BC:2 * BC], r0_sb)
                        else:
                            nc.vector.tensor_scalar_mul(mean, pst[:, 0:BC], 1.0 / WIN)
                            nc.vector.tensor_scalar_mul(mean2, pst[:, BC:2 * BC], 1.0 / WIN)
                        msq = spool.tile([PCH, BC], F32, tag="msq")
                        nc.scalar.activation(out=msq, in_=mean, func=AF.Square)
                        var = spool.tile([PCH, BC], F32, tag="var")
                        nc.vector.tensor_sub(var, mean2, msq)
                        nc.vector.tensor_scalar_max(var, var, 0.0)
                        lv = spool.tile([PCH, BC], F16, tag="lv")
                        nc.scalar.activation(
                            out=lv, in_=var, func=AF.Ln, bias=bias_sb[:, 0:1])
                        rsq = spool.tile([PCH, BC], F32, tag="rsq")
                        nc.scalar.activation(out=rsq, in_=lv, func=AF.Exp, scale=-0.5,
                                             bias=bias_sb[:, 6:7])
                        xm = spool.tile([PCH, BC], F32, tag="xm")
                        nc.vector.tensor_sub(xm, xt, mean)
                        zt = spool.tile([PCH, BC], F16, tag="zt")
                        nc.vector.tensor_mul(zt, xm, rsq)
                        nc.gpsimd.dma_start(
                            out=zl_d[ch * 32:(ch + 1) * 32, :, 0, :]
                                .rearrange("p4 m bc -> (p4 m) bc"),
                            in_=zt)
                        nc.gpsimd.dma_start(
                            out=zl_d[ch * 32:(ch + 1) * 32, :, 1, :]
                                .rearrange("p4 m bc -> (p4 m) bc"),
                            in_=lv)
                        if ch == NCH - 1:
                            nc.gpsimd.dma_start(
                                out=zl_d[1024:1056, :, 0, :]
                                    .rearrange("p4 m bc -> (p4 m) bc"),
                                in_=zt)
                            nc.gpsimd.dma_start(
                                out=zl_d[1024:1056, :, 1, :]
                                    .rearrange("p4 m bc -> (p4 m) bc"),
                                in_=lv)
                        xprev_r, x2prev = xtr, x2

                # ---------------- Phase B ----------------
                if phases < 2:
                    phases_skip = True
                else:
                    phases_skip = False
                with tc.tile_pool(name="x1p", bufs=3) as x1pool, \
                     tc.tile_pool(name="hps", bufs=2, space="PSUM") as hpsum, \
                     tc.tile_pool(name="hsb", bufs=3) as hpool, \
                     tc.tile_pool(name="l2ps", bufs=2, space="PSUM") as l2psum, \
                     tc.tile_pool(name="lcp", bufs=4) as lcpool:
                    for up in range(88 if not phases_skip else 0):  # unit pairs: 48 positions each
                        x1 = x1pool.tile([8, 12 * BC], F16, tag="x1")
                        nc.sync.dma_start(
                            out=x1.rearrange("mf (p4 bc) -> mf p4 bc", p4=12),
                            in_=zl_d[12 * up:12 * up + 12]
                                .rearrange("p4 m f bc -> (m f) p4 bc"))
                        l2 = l2psum.tile([120, 512], F32, tag="l2")
                        for ul in range(2):
                            hp = hpsum.tile([128, 6 * BC], F32, tag="h")
                            for a in range(6):
                                blk = ul * 6 + a
                                nc.tensor.matmul(
                                    hp[:, a * BC:(a + 1) * BC],
                                    lhsT=w1_sbr,
                                    rhs=x1[:, blk * BC:(blk + 1) * BC],
                                    start=True, stop=True)
                            hs = hpool.tile([128, 6 * BC], F32R, tag="hs")
                            nc.scalar.activation(out=hs, in_=hp, func=AF.Gelu, bias=b1_sb)
                            for a in range(6):
                                nc.tensor.matmul(
                                    l2[:, ul * BC:(ul + 1) * BC],
                                    lhsT=w2_sbr[:, a * 120:(a + 1) * 120],
                                    rhs=hs[:, a * BC:(a + 1) * BC],
                                    start=(a == 0), stop=(a == 5))
                        lcp = lcpool.tile([120, 512], F16, tag="lc")
                        nc.vector.tensor_copy(lcp, l2)
                        for ul in range(2):
                            p4a = 12 * up + 6 * ul
                            nc.gpsimd.dma_start(
                                out=lg_d[p4a:p4a + 6, :, :, :]
                                    .rearrange("p4 m k bc -> (p4 m k) bc"),
                                in_=lcp[:, ul * BC:(ul + 1) * BC])

                # ---------------- Phase C/D ----------------
                tc.no_sync_barrier()
                NSC = 8 if phases >= 3 else 0
                SCW = 4 * BC  # 1024
                with tc.tile_pool(name="lgp", bufs=2) as lpool, \
                     tc.tile_pool(name="ep", bufs=2) as epool, \
                     tc.tile_pool(name="dp", bufs=3) as dpool, \
                     tc.tile_pool(name="op", bufs=3) as opool:
                    for sc in range(NSC):
                        et = epool.tile([PCH, NK * SCW], BF16, tag="e")
                        lgt = lpool.tile([PCH, NK * SCW], F16, tag="lg")
                        for k in range(NK):
                            nc.gpsimd.dma_start(
                                out=lgt[:, k * SCW:(k + 1) * SCW]
                                    .rearrange("p (q bc) -> p q bc", q=4),
                                in_=lg_d[sc * 128:(sc + 1) * 128, :, k, :]
                                    .rearrange("(q p4) m bc -> (p4 m) q bc", q=4))
                        if sc == 0:
                            # only t<16 can overflow exp: per-element max-subtract here
                            mx = lpool.tile([PCH, SCW], F16, tag="mx")
                            nc.vector.tensor_max(mx, lgt[:, 0:SCW], lgt[:, SCW:2 * SCW])
                            for k in (2, 3, 4):
                                nc.vector.tensor_max(mx, mx, lgt[:, k * SCW:(k + 1) * SCW])
                            for k in range(NK):
                                dk = lpool.tile([PCH, SCW], F16, tag="dk")
                                nc.vector.tensor_sub(dk, lgt[:, k * SCW:(k + 1) * SCW], mx)
                                nc.scalar.activation(
                                    out=et[:, k * SCW:(k + 1) * SCW], in_=dk,
                                    func=AF.Exp, scale=inv_t, bias=bias_sb[:, 1 + k:2 + k])
                        else:
                            for k in range(NK):
                                nc.scalar.activation(
                                    out=et[:, k * SCW:(k + 1) * SCW],
                                    in_=lgt[:, k * SCW:(k + 1) * SCW],
                                    func=AF.Exp, scale=inv_t, bias=bias_sb[:, 1 + k:2 + k])
                        den = dpool.tile([PCH, SCW], BF16, tag="den")
                        nc.vector.tensor_add(den, et[:, 0:SCW], et[:, SCW:2 * SCW])
                        for k in (2, 3, 4):
                            nc.vector.tensor_add(den, den, et[:, k * SCW:(k + 1) * SCW])
                        ld = dpool.tile([PCH, SCW], F32, tag="ld")
                        nc.scalar.activation(out=ld, in_=den, func=AF.Ln)
                        rd = dpool.tile([PCH, SCW], BF16, tag="rd")
                        nc.scalar.activation(out=rd, in_=ld, func=AF.Exp, scale=-1.0)
                        yv = y_all.rearrange("p (ch k bc) -> p ch k bc", ch=NCH, k=NK)
                        ysl = lambda k: yv[:, 4 * sc:4 * sc + 4, k, :]
                        num = dpool.tile([PCH, SCW], BF16, tag="num")
                        tmp = dpool.tile([PCH, SCW], BF16, tag="tmp")
                        nc.vector.tensor_mul(num, et[:, 0:SCW], ysl(0))
                        for k in range(1, NK):
                            nc.vector.tensor_mul(tmp, et[:, k * SCW:(k + 1) * SCW], ysl(k))
                            nc.vector.tensor_add(num, num, tmp)
                        ot = opool.tile([PCH, SCW], F32, tag="ot")
                        nc.vector.tensor_mul(ot, num, rd)
                        for q in range(4):
                            nc.sync.dma_start(
                                out=out_d[:, sc * 512 + q * 128:sc * 512 + (q + 1) * 128, :]
                                    .rearrange("b p c -> p b c"),
                                in_=ot[:, q * BC:(q + 1) * BC]
                                    .rearrange("p (b c) -> p b c", b=BL))
    nc.compile()
    return nc


def kernel(x, W1, b1, W2, b2, _trace=False, _dbg=False):
    global LAST_EXEC_NS, LAST_RESULTS
    x = np.ascontiguousarray(np.asarray(x, np.float32))
    W1 = np.asarray(W1, np.float32)
    b1 = np.asarray(b1, np.float32)
    W2 = np.asarray(W2, np.float32)
    b2 = np.asarray(b2, np.float32)

    convm, w1blk, b1vec, w2blk, r0b = _build_consts(W1, b1, W2)
    convm = np.ascontiguousarray(convm.transpose(1, 0, 2))  # [pi, i, po]
    bvals = np.concatenate([[np.float32(1e-6)],
                            (b2 / np.float32(TEMP)).astype(np.float32),
                            [np.float32(np.log(0.25))]])
    biasv = np.ascontiguousarray(np.broadcast_to(bvals[None, :], (PCH, 7)).astype(np.float32))
    nc = _build_graph(b2, dbg=_dbg)

    in_maps = []
    for i in range(NCORES):
        xs = x[i * BL:(i + 1) * BL]
        x0rep = np.broadcast_to(
            xs[:, 0, :].reshape(1, BC), (PCH, BC)).copy().astype(np.float32)
        in_maps.append({
            "x": np.ascontiguousarray(xs),
            "convmr": convm,
            "w1blkr": w1blk.astype(np.float16), "b1vec": b1vec,
            "w2blkr": w2blk, "r0b": r0b,
            "x0rep": x0rep, "x0repr": x0rep,
            "biasv": biasv,
        })
    res = run_bass_kernel_spmd(nc, in_maps, list(range(NCORES)), trace=_trace)
    LAST_EXEC_NS = res.exec_time_ns
    LAST_RESULTS = res
    out = np.concatenate([res.results[i]["out"] for i in range(NCORES)], axis=0)
    return out.astype(np.float32)


if __name__ == "__main__":
    rng = np.random.default_rng(0)
    xs = rng.standard_normal((B, T, C)).astype(np.float32)
    W1 = rng.standard_normal((H, 2)).astype(np.float32) * 0.5
    b1 = rng.standard_normal((H,)).astype(np.float32) * 0.1
    W2 = rng.standard_normal((NK, H)).astype(np.float32) * 0.3
    b2 = rng.standard_normal((NK,)).astype(np.float32) * 0.1
    o = kernel(xs, W1, b1, W2, b2)
    print("ran", o.shape, o.dtype)



# revision 18
# speedup vs baseline: 1.0928x; 1.0372x over previous
"""AdaptiveGaussianTrendCausal Trainium2 kernel (8-core data parallel).

Strategy (per core, B_loc=4, T=4096, C=64; elements laid out as
[128 position partitions x 256 (b,c) free] chunk tiles):

Phase A (ScalarE table set: natural_log_exp):
  - causal depthwise convs as banded-Toeplitz matmuls (cur+prev chunk
    accumulated in PSUM). 5 gaussian filters in f32r, box sums (win=16)
    for running stats in exact f32 (variance cancellation safety).
  - running mean/var on VectorE, logv = Ln(var+1e-6),
    rsqrt via exp(-0.5*logv)  (avoids sqrt table set entirely),
    z = (x-mean)*rsq.  Y_k saved to SBUF in bf16 via ScalarE copies.

Phase B (table set: gelu):
  - DMA-rearrange z/logv into blockdiag moving layout [8, .] (4 elems/col)
  - L1 blockdiag matmul -> PSUM h [128 = 4 elems x 32 hidden, .]
  - exact Gelu with per-partition bias b1 -> SBUF
  - L2 blockdiag matmul -> logits PSUM [20 = 4 elems x 5 k, .]
  - DMA logits to DRAM scratch in a k-plane friendly layout

Phase C/D (table set: natural_log_exp):
  - reload logits as full-partition k-plane chunks
  - e_k = Exp(logits/0.7 + b2_k/0.7)   (bias folded into activation)
    no max-subtract needed: max |logit|/0.7 ~ 42 << 88 (fp32 exp limit)
  - den = sum_k e_k, rden = exp(-ln(den)), num = sum_k Y_k*e_k (bf16 DVE)
  - trend = num * rden -> DMA out
"""

import os
import sys
import numpy as np

for _p in ("/opt/trn_rl_repo",):
    if _p not in sys.path and os.path.isdir(_p):
        sys.path.insert(0, _p)

from concourse import bass, mybir
from concourse import bacc

# Constrain the activation-table chooser: serve ln/exp only from the combined
# natural_log_exp set (and keep gelu's set pure) so phase-interleaved ACT
# streams don't ping-pong table loads. List order/IDs are preserved.
import concourse.hw_specs as _hw_specs
_orig_get_tables = _hw_specs.get_activation_tables
def _pinned_tables(module_arch):
    tabs = _orig_get_tables(module_arch)
    out = {}
    for name, funcs in tabs.items():
        f = set(funcs)
        if name not in ("natural_log_exp_and_others",):
            f.discard(mybir.ActivationFunctionType.Ln)
            f.discard(mybir.ActivationFunctionType.Exp)
        out[name] = f
    return out
_hw_specs.get_activation_tables = _pinned_tables
bacc.get_activation_tables = _pinned_tables
from concourse.tile import TileContext
from concourse.bass_utils import run_bass_kernel_spmd

F32 = mybir.dt.float32
F32R = mybir.dt.float32r
BF16 = mybir.dt.bfloat16
F16 = mybir.dt.float16
AF = mybir.ActivationFunctionType

B, T, C, H, NK = 32, 4096, 64, 32, 5
NCORES = 8
BL = B // NCORES          # 4 batch elems per core
BC = BL * C               # 256 free columns per chunk
PCH = 128                 # positions per chunk
NCH = T // PCH            # 32 chunks
SIGMAS = (2.5, 4.0, 6.0, 9.0, 14.0)
WIN = 16
TEMP = 0.7

LAST_EXEC_NS = None
LAST_RESULTS = None


def _gauss_kernel_np(sigma):
    # matches reference._gauss_kernel in float32 arithmetic
    R = min(max(1, int(4.0 * sigma + 0.5)), max(1, T - 1))
    n = np.arange(0, R + 1, dtype=np.float32)
    k = np.exp(-0.5 * (n / np.float32(max(float(sigma), 1e-6))) ** 2).astype(np.float32)
    return (k / (k.sum() + np.float32(1e-12))).astype(np.float32)


def _band_mats(k):
    """Toeplitz pair (Acur, Aprev) with out = Acur.T@x_cur + Aprev.T@x_prev.

    Acur[pi, po] = k[po - pi]        for 0 <= po-pi <= R
    Aprev[pi, po] = k[po + 128 - pi] for 1 <= po+128-pi <= R
    (chunk0 uses x_prev = x[0] replicated -> exactly the edge padding)
    """
    R = len(k) - 1
    cur = np.zeros((PCH, PCH), np.float32)
    prv = np.zeros((PCH, PCH), np.float32)
    for pi in range(PCH):
        for po in range(PCH):
            d = po - pi
            if 0 <= d <= R:
                cur[pi, po] = k[d]
            d2 = po + PCH - pi
            if 1 <= d2 <= R:
                prv[pi, po] = k[d2]
    return cur, prv


def _build_consts(W1, b1, W2):
    convm = np.zeros((12, PCH, PCH), np.float32)
    for f, s in enumerate(SIGMAS):
        cur, prv = _band_mats(_gauss_kernel_np(s)[::-1].copy())
        convm[2 * f] = cur
        convm[2 * f + 1] = prv
    bcur, bprv = _band_mats(np.ones(WIN, np.float32))
    convm[10] = bcur
    convm[11] = bprv

    w1blk = np.zeros((8, 128), np.float32)
    b1vec = np.zeros((128, 1), np.float32)
    w2blk = np.zeros((128, 6 * 120), np.float32)
    for m in range(4):
        for j in range(H):
            w1blk[2 * m, 32 * m + j] = W1[j, 0] * 4.0
            w1blk[2 * m + 1, 32 * m + j] = W1[j, 1]
            b1vec[32 * m + j, 0] = b1[j]
            for a in range(6):
                for k in range(NK):
                    w2blk[32 * m + j, 120 * a + 20 * a + 5 * m + k] = W2[k, j] / 16.0

    eff = np.minimum(np.arange(1, PCH + 1, dtype=np.float32), np.float32(WIN))
    r0 = (np.float32(1.0) / (eff + np.float32(1e-12))).astype(np.float32)
    r0b = np.broadcast_to(r0[:, None], (PCH, BC)).copy()
    return convm, w1blk, b1vec, w2blk, r0b


def _build_graph(b2, dbg=False, phases=3):
    nc = bacc.Bacc()
    x_d = nc.declare_dram_parameter("x", [BL, T, C], F32, isOutput=False)
    convr_d = nc.declare_dram_parameter("convmr", [PCH, 12, PCH], F32R, isOutput=False)
    w1r_d = nc.declare_dram_parameter("w1blkr", [8, 128], F16, isOutput=False)
    w2r_d = nc.declare_dram_parameter("w2blkr", [128, 6 * 120], F32R, isOutput=False)
    x0r_d = nc.declare_dram_parameter("x0repr", [PCH, BC], F32R, isOutput=False)
    b1_d = nc.declare_dram_parameter("b1vec", [128, 1], F32, isOutput=False)
    r0_d = nc.declare_dram_parameter("r0b", [PCH, BC], F32, isOutput=False)
    x0_d = nc.declare_dram_parameter("x0rep", [PCH, BC], F32, isOutput=False)
    bias_d = nc.declare_dram_parameter("biasv", [PCH, 7], F32, isOutput=False)
    out_d = nc.declare_dram_parameter("out", [BL, T, C], F32, isOutput=True)
    # logits scratch [k, p4, m, bc]
    TP = 4224  # T padded to a multiple of 384 (48-pos units x 128-pos chunks)
    if dbg:
        lg_d = nc.declare_dram_parameter("lgdbg", [TP // 4, 4, NK, BC], F16, isOutput=True)
        zl_d = nc.declare_dram_parameter("zldbg", [TP // 4, 4, 2, BC], F16, isOutput=True)
    else:
        lg_d = nc.dram_tensor("lgscratch", [TP // 4, 4, NK, BC], F16)
        zl_d = nc.dram_tensor("zlscratch", [TP // 4, 4, 2, BC], F16)

    inv_t = float(16.0 / TEMP)

    with TileContext(nc) as tc:
        with tc.tile_pool(name="const", bufs=1) as cpool:

            b1_sb = cpool.tile([128, 1], F32)
            nc.sync.dma_start(out=b1_sb, in_=b1_d[:])
            r0_sb = cpool.tile([PCH, BC], F32)
            nc.sync.dma_start(out=r0_sb, in_=r0_d[:])
            x0_sb = cpool.tile([PCH, BC], F32)
            nc.sync.dma_start(out=x0_sb, in_=x0_d[:])
            x02_sb = cpool.tile([PCH, BC], F32R)
            bias_sb = cpool.tile([PCH, 7], F32)
            nc.sync.dma_start(out=bias_sb, in_=bias_d[:])
            conv_sbr = cpool.tile([PCH, 12 * PCH], F32R)
            nc.sync.dma_start(out=conv_sbr, in_=convr_d.rearrange("p i q -> p (i q)"))
            w1_sbr = cpool.tile([8, 128], F16)
            nc.sync.dma_start(out=w1_sbr, in_=w1r_d[:])
            w2_sbr = cpool.tile([128, 6 * 120], F32R)
            nc.sync.dma_start(out=w2_sbr, in_=w2r_d[:])
            x0_sbr = cpool.tile([PCH, BC], F32R)
            nc.sync.dma_start(out=x0_sbr, in_=x0r_d[:])


            with tc.tile_pool(name="feat", bufs=1) as fpool:
                y_all = fpool.tile([PCH, NCH * NK * BC], BF16)  # free = (ch, k, bc)

                # ---------------- Phase A ----------------
                with tc.tile_pool(name="xp", bufs=6) as xpool, \
                     tc.tile_pool(name="x2p", bufs=4) as x2pool, \
                     tc.tile_pool(name="cps", bufs=2, space="PSUM") as cpsum, \
                     tc.tile_pool(name="stat", bufs=6) as spool:
                    nc.vector.tensor_mul(x02_sb, x0_sb, x0_sb)
                    xprev_r, x2prev = x0_sbr, x02_sb
                    for ch in range(NCH):
                        xt = xpool.tile([PCH, BC], F32, tag="x")
                        nc.sync.dma_start(
                            out=xt.rearrange("p (b c) -> p b c", b=BL),
                            in_=x_d[:, ch * PCH:(ch + 1) * PCH, :]
                                .rearrange("b p c -> p b c"))
                        xtr = xpool.tile([PCH, BC], F32R, tag="xr")
                        nc.vector.tensor_copy(xtr, xt)
                        x2 = x2pool.tile([PCH, BC], F32R, tag="x2")
                        nc.vector.tensor_mul(x2, xt, xt)
                        psg = cpsum.tile([PCH, 5 * BC], F32, tag="cvg")
                        pst = cpsum.tile([PCH, 2 * BC], F32, tag="cvs")
                        for f in range(5):
                            nc.tensor.matmul(
                                psg[:, f * BC:(f + 1) * BC],
                                lhsT=conv_sbr[:, (2 * f) * PCH:(2 * f + 1) * PCH],
                                rhs=xtr, start=True, stop=False)
                            nc.tensor.matmul(
                                psg[:, f * BC:(f + 1) * BC],
                                lhsT=conv_sbr[:, (2 * f + 1) * PCH:(2 * f + 2) * PCH],
                                rhs=xprev_r, start=False, stop=True)
                        nc.tensor.matmul(pst[:, 0:BC],
                                         lhsT=conv_sbr[:, 10 * PCH:11 * PCH],
                                         rhs=xtr, start=True, stop=False)
                        nc.tensor.matmul(pst[:, 0:BC],
                                         lhsT=conv_sbr[:, 11 * PCH:12 * PCH],
                                         rhs=xprev_r, start=False, stop=True)
                        nc.tensor.matmul(pst[:, BC:2 * BC],
                                         lhsT=conv_sbr[:, 10 * PCH:11 * PCH],
                                         rhs=x2, start=True, stop=False)
                        nc.tensor.matmul(pst[:, BC:2 * BC],
                                         lhsT=conv_sbr[:, 11 * PCH:12 * PCH],
                                         rhs=x2prev, start=False, stop=True)
                        nc.scalar.activation(
                            out=y_all[:, ch * NK * BC:(ch + 1) * NK * BC],
                            in_=psg, func=AF.Copy)
                        mean = spool.tile([PCH, BC], F32, tag="mean")
                        mean2 = spool.tile([PCH, BC], F32, tag="mean2")
                        if ch == 0:
                            nc.vector.tensor_mul(mean, pst[:, 0:BC], r0_sb)
                            nc.vector.tensor_mul(mean2, pst[:, BC:2 * BC], r0_sb)
                        else:
                            nc.vector.tensor_scalar_mul(mean, pst[:, 0:BC], 1.0 / WIN)
                            nc.vector.tensor_scalar_mul(mean2, pst[:, BC:2 * BC], 1.0 / WIN)
                        msq = spool.tile([PCH, BC], F32, tag="msq")
                        nc.scalar.activation(out=msq, in_=mean, func=AF.Square)
                        var = spool.tile([PCH, BC], F32, tag="var")
                        nc.vector.tensor_sub(var, mean2, msq)
                        nc.vector.tensor_scalar_max(var, var, 0.0)
                        lv = spool.tile([PCH, BC], F16, tag="lv")
                        nc.scalar.activation(
                            out=lv, in_=var, func=AF.Ln, bias=bias_sb[:, 0:1])
                        rsq = spool.tile([PCH, BC], F32, tag="rsq")
                        nc.scalar.activation(out=rsq, in_=lv, func=AF.Exp, scale=-0.5,
                                             bias=bias_sb[:, 6:7])
                        xm = spool.tile([PCH, BC], F32, tag="xm")
                        nc.vector.tensor_sub(xm, xt, mean)
                        zt = spool.tile([PCH, BC], F16, tag="zt")
                        nc.vector.tensor_mul(zt, xm, rsq)
                        nc.gpsimd.dma_start(
                            out=zl_d[ch * 32:(ch + 1) * 32, :, 0, :]
                                .rearrange("p4 m bc -> (p4 m) bc"),
                            in_=zt)
                        nc.gpsimd.dma_start(
                            out=zl_d[ch * 32:(ch + 1) * 32, :, 1, :]
                                .rearrange("p4 m bc -> (p4 m) bc"),
                            in_=lv)
                        if ch == NCH - 1:
                            nc.gpsimd.dma_start(
                                out=zl_d[1024:1056, :, 0, :]
                                    .rearrange("p4 m bc -> (p4 m) bc"),
                                in_=zt)
                            nc.gpsimd.dma_start(
                                out=zl_d[1024:1056, :, 1, :]
                                    .rearrange("p4 m bc -> (p4 m) bc"),
                                in_=lv)
                        xprev_r, x2prev = xtr, x2

                # ---------------- Phase B ----------------
                if phases < 2:
                    phases_skip = True
                else:
                    phases_skip = False
                with tc.tile_pool(name="x1p", bufs=3) as x1pool, \
                     tc.tile_pool(name="hps", bufs=2, space="PSUM") as hpsum, \
                     tc.tile_pool(name="hsb", bufs=3) as hpool, \
                     tc.tile_pool(name="l2ps", bufs=2, space="PSUM") as l2psum, \
                     tc.tile_pool(name="lcp", bufs=4) as lcpool:
                    for up in range(88 if not phases_skip else 0):  # unit pairs: 48 positions each
                        x1 = x1pool.tile([8, 12 * BC], F16, tag="x1")
                        nc.sync.dma_start(
                            out=x1.rearrange("mf (p4 bc) -> mf p4 bc", p4=12),
                            in_=zl_d[12 * up:12 * up + 12]
                                .rearrange("p4 m f bc -> (m f) p4 bc"))
                        l2 = l2psum.tile([120, 512], F32, tag="l2")
                        for ul in range(2):
                            hp = hpsum.tile([128, 6 * BC], F32, tag="h")
                            for a in range(6):
                                blk = ul * 6 + a
                                nc.tensor.matmul(
                                    hp[:, a * BC:(a + 1) * BC],
                                    lhsT=w1_sbr,
                                    rhs=x1[:, blk * BC:(blk + 1) * BC],
                                    start=True, stop=True)
                            hs = hpool.tile([128, 6 * BC], F32R, tag="hs")
                            nc.scalar.activation(out=hs, in_=hp, func=AF.Gelu, bias=b1_sb)
                            for a in range(6):
                                nc.tensor.matmul(
                                    l2[:, ul * BC:(ul + 1) * BC],
                                    lhsT=w2_sbr[:, a * 120:(a + 1) * 120],
                                    rhs=hs[:, a * BC:(a + 1) * BC],
                                    start=(a == 0), stop=(a == 5))
                        lcp = lcpool.tile([120, 512], F16, tag="lc")
                        nc.vector.tensor_copy(lcp, l2)
                        for ul in range(2):
                            p4a = 12 * up + 6 * ul
                            nc.gpsimd.dma_start(
                                out=lg_d[p4a:p4a + 6, :, :, :]
                                    .rearrange("p4 m k bc -> (p4 m k) bc"),
                                in_=lcp[:, ul * BC:(ul + 1) * BC])

                # ---------------- Phase C/D ----------------
                tc.no_sync_barrier()
                NSC = 8 if phases >= 3 else 0
                SCW = 4 * BC  # 1024
                with tc.tile_pool(name="lgp", bufs=2) as lpool, \
                     tc.tile_pool(name="ep", bufs=2) as epool, \
                     tc.tile_pool(name="dp", bufs=3) as dpool, \
                     tc.tile_pool(name="op", bufs=3) as opool:
                    for sc in range(NSC):
                        et = epool.tile([PCH, NK * SCW], BF16, tag="e")
                        lgt = lpool.tile([PCH, NK * SCW], F16, tag="lg")
                        for k in range(NK):
                            nc.gpsimd.dma_start(
                                out=lgt[:, k * SCW:(k + 1) * SCW]
                                    .rearrange("p (q bc) -> p q bc", q=4),
                                in_=lg_d[sc * 128:(sc + 1) * 128, :, k, :]
                                    .rearrange("(q p4) m bc -> (p4 m) q bc", q=4))
                        if sc == 0:
                            # only t<16 can overflow exp: per-element max-subtract here
                            mx = lpool.tile([PCH, SCW], F16, tag="mx")
                            nc.vector.tensor_max(mx, lgt[:, 0:SCW], lgt[:, SCW:2 * SCW])
                            for k in (2, 3, 4):
                                nc.vector.tensor_max(mx, mx, lgt[:, k * SCW:(k + 1) * SCW])
                            for k in range(NK):
                                dk = lpool.tile([PCH, SCW], F16, tag="dk")
                                nc.vector.tensor_sub(dk, lgt[:, k * SCW:(k + 1) * SCW], mx)
                                nc.scalar.activation(
                                    out=et[:, k * SCW:(k + 1) * SCW], in_=dk,
                                    func=AF.Exp, scale=inv_t, bias=bias_sb[:, 1 + k:2 + k])
                        else:
                            for k in range(NK):
                                nc.scalar.activation(
                                    out=et[:, k * SCW:(k + 1) * SCW],
                                    in_=lgt[:, k * SCW:(k + 1) * SCW],
                                    func=AF.Exp, scale=inv_t, bias=bias_sb[:, 1 + k:2 + k])
                        den = dpool.tile([PCH, SCW], BF16, tag="den")
                        nc.vector.tensor_add(den, et[:, 0:SCW], et[:, SCW:2 * SCW])
                        for k in (2, 3, 4):
                            nc.vector.tensor_add(den, den, et[:, k * SCW:(k + 1) * SCW])
                        ld = dpool.tile([PCH, SCW], F32, tag="ld")
                        nc.scalar.activation(out=ld, in_=den, func=AF.Ln)
                        rd = dpool.tile([PCH, SCW], BF16, tag="rd")
                        nc.scalar.activation(out=rd, in_=ld, func=AF.Exp, scale=-1.0)
                        yv = y_all.rearrange("p (ch k bc) -> p ch k bc", ch=NCH, k=NK)
                        ysl = lambda k: yv[:, 4 * sc:4 * sc + 4, k, :]
                        num = dpool.tile([PCH, SCW], BF16, tag="num")
                        tmp = dpool.tile([PCH, SCW], BF16, tag="tmp")
                        nc.vector.tensor_mul(num, et[:, 0:SCW], ysl(0))
                        for k in range(1, NK):
                            nc.vector.tensor_mul(tmp, et[:, k * SCW:(k + 1) * SCW], ysl(k))
                            nc.vector.tensor_add(num, num, tmp)
                        ot = opool.tile([PCH, SCW], F32, tag="ot")
                        nc.gpsimd.tensor_mul(ot, num, rd)
                        for q in range(4):
                            nc.sync.dma_start(
                                out=out_d[:, sc * 512 + q * 128:sc * 512 + (q + 1) * 128, :]
                                    .rearrange("b p c -> p b c"),
                                in_=ot[:, q * BC:(q + 1) * BC]
                                    .rearrange("p (b c) -> p b c", b=BL))
    nc.compile()
    return nc


def kernel(x, W1, b1, W2, b2, _trace=False, _dbg=False):
    global LAST_EXEC_NS, LAST_RESULTS
    x = np.ascontiguousarray(np.asarray(x, np.float32))
    W1 = np.asarray(W1, np.float32)
    b1 = np.asarray(b1, np.float32)
    W2 = np.asarray(W2, np.float32)
    b2 = np.asarray(b2, np.float32)

    convm, w1blk, b1vec, w2blk, r0b = _build_consts(W1, b1, W2)
    convm = np.ascontiguousarray(convm.transpose(1, 0, 2))  # [pi, i, po]
    bvals = np.concatenate([[np.float32(1e-6)],
                            (b2 / np.float32(TEMP)).astype(np.float32),
                            [np.float32(np.log(0.25))]])
    biasv = np.ascontiguousarray(np.broadcast_to(bvals[None, :], (PCH, 7)).astype(np.float32))
    nc = _build_graph(b2, dbg=_dbg)

    in_maps = []
    for i in range(NCORES):
        xs = x[i * BL:(i + 1) * BL]
        x0rep = np.broadcast_to(
            xs[:, 0, :].reshape(1, BC), (PCH, BC)).copy().astype(np.float32)
        in_maps.append({
            "x": np.ascontiguousarray(xs),
            "convmr": convm,
            "w1blkr": w1blk.astype(np.float16), "b1vec": b1vec,
            "w2blkr": w2blk, "r0b": r0b,
            "x0rep": x0rep, "x0repr": x0rep,
            "biasv": biasv,
        })
    res = run_bass_kernel_spmd(nc, in_maps, list(range(NCORES)), trace=_trace)
    LAST_EXEC_NS = res.exec_time_ns
    LAST_RESULTS = res
    out = np.concatenate([res.results[i]["out"] for i in range(NCORES)], axis=0)
    return out.astype(np.float32)


if __name__ == "__main__":
    rng = np.random.default_rng(0)
    xs = rng.standard_normal((B, T, C)).astype(np.float32)
    W1 = rng.standard_normal((H, 2)).astype(np.float32) * 0.5
    b1 = rng.standard_normal((H,)).astype(np.float32) * 0.1
    W2 = rng.standard_normal((NK, H)).astype(np.float32) * 0.3
    b2 = rng.standard_normal((NK,)).astype(np.float32) * 0.1
    o = kernel(xs, W1, b1, W2, b2)
    print("ran", o.shape, o.dtype)



# revision 24
# speedup vs baseline: 1.0962x; 1.0031x over previous
"""AdaptiveGaussianTrendCausal Trainium2 kernel (8-core data parallel).

Strategy (per core, B_loc=4, T=4096, C=64; elements laid out as
[128 position partitions x 256 (b,c) free] chunk tiles):

Phase A (ScalarE table set: natural_log_exp):
  - causal depthwise convs as banded-Toeplitz matmuls (cur+prev chunk
    accumulated in PSUM). 5 gaussian filters in f32r, box sums (win=16)
    for running stats in exact f32 (variance cancellation safety).
  - running mean/var on VectorE, logv = Ln(var+1e-6),
    rsqrt via exp(-0.5*logv)  (avoids sqrt table set entirely),
    z = (x-mean)*rsq.  Y_k saved to SBUF in bf16 via ScalarE copies.

Phase B (table set: gelu):
  - DMA-rearrange z/logv into blockdiag moving layout [8, .] (4 elems/col)
  - L1 blockdiag matmul -> PSUM h [128 = 4 elems x 32 hidden, .]
  - exact Gelu with per-partition bias b1 -> SBUF
  - L2 blockdiag matmul -> logits PSUM [20 = 4 elems x 5 k, .]
  - DMA logits to DRAM scratch in a k-plane friendly layout

Phase C/D (table set: natural_log_exp):
  - reload logits as full-partition k-plane chunks
  - e_k = Exp(logits/0.7 + b2_k/0.7)   (bias folded into activation)
    no max-subtract needed: max |logit|/0.7 ~ 42 << 88 (fp32 exp limit)
  - den = sum_k e_k, rden = exp(-ln(den)), num = sum_k Y_k*e_k (bf16 DVE)
  - trend = num * rden -> DMA out
"""

import os
import sys
import numpy as np

for _p in ("/opt/trn_rl_repo",):
    if _p not in sys.path and os.path.isdir(_p):
        sys.path.insert(0, _p)

from concourse import bass, mybir
from concourse import bacc

# Constrain the activation-table chooser: serve ln/exp only from the combined
# natural_log_exp set (and keep gelu's set pure) so phase-interleaved ACT
# streams don't ping-pong table loads. List order/IDs are preserved.
import concourse.hw_specs as _hw_specs
_orig_get_tables = _hw_specs.get_activation_tables
def _pinned_tables(module_arch):
    tabs = _orig_get_tables(module_arch)
    out = {}
    for name, funcs in tabs.items():
        f = set(funcs)
        if name not in ("natural_log_exp_and_others",):
            f.discard(mybir.ActivationFunctionType.Ln)
            f.discard(mybir.ActivationFunctionType.Exp)
        out[name] = f
    return out
_hw_specs.get_activation_tables = _pinned_tables
bacc.get_activation_tables = _pinned_tables
from concourse.tile import TileContext
from concourse.bass_utils import run_bass_kernel_spmd

F32 = mybir.dt.float32
F32R = mybir.dt.float32r
BF16 = mybir.dt.bfloat16
F16 = mybir.dt.float16
AF = mybir.ActivationFunctionType

B, T, C, H, NK = 32, 4096, 64, 32, 5
NCORES = 8
BL = B // NCORES          # 4 batch elems per core
BC = BL * C               # 256 free columns per chunk
PCH = 128                 # positions per chunk
NCH = T // PCH            # 32 chunks
SIGMAS = (2.5, 4.0, 6.0, 9.0, 14.0)
WIN = 16
TEMP = 0.7

LAST_EXEC_NS = None
LAST_RESULTS = None


def _gauss_kernel_np(sigma):
    # matches reference._gauss_kernel in float32 arithmetic
    R = min(max(1, int(4.0 * sigma + 0.5)), max(1, T - 1))
    n = np.arange(0, R + 1, dtype=np.float32)
    k = np.exp(-0.5 * (n / np.float32(max(float(sigma), 1e-6))) ** 2).astype(np.float32)
    return (k / (k.sum() + np.float32(1e-12))).astype(np.float32)


def _band_mats(k):
    """Toeplitz pair (Acur, Aprev) with out = Acur.T@x_cur + Aprev.T@x_prev.

    Acur[pi, po] = k[po - pi]        for 0 <= po-pi <= R
    Aprev[pi, po] = k[po + 128 - pi] for 1 <= po+128-pi <= R
    (chunk0 uses x_prev = x[0] replicated -> exactly the edge padding)
    """
    R = len(k) - 1
    cur = np.zeros((PCH, PCH), np.float32)
    prv = np.zeros((PCH, PCH), np.float32)
    for pi in range(PCH):
        for po in range(PCH):
            d = po - pi
            if 0 <= d <= R:
                cur[pi, po] = k[d]
            d2 = po + PCH - pi
            if 1 <= d2 <= R:
                prv[pi, po] = k[d2]
    return cur, prv


def _build_consts(W1, b1, W2):
    convm = np.zeros((12, PCH, PCH), np.float32)
    for f, s in enumerate(SIGMAS):
        cur, prv = _band_mats(_gauss_kernel_np(s)[::-1].copy())
        convm[2 * f] = cur
        convm[2 * f + 1] = prv
    bcur, bprv = _band_mats(np.ones(WIN, np.float32))
    convm[10] = bcur
    convm[11] = bprv

    w1blk = np.zeros((8, 128), np.float32)
    b1vec = np.zeros((128, 1), np.float32)
    w2blk = np.zeros((128, 6 * 120), np.float32)
    for m in range(4):
        for j in range(H):
            w1blk[2 * m, 32 * m + j] = W1[j, 0] * 4.0
            w1blk[2 * m + 1, 32 * m + j] = W1[j, 1]
            b1vec[32 * m + j, 0] = b1[j]
            for a in range(6):
                for k in range(NK):
                    w2blk[32 * m + j, 120 * a + 20 * a + 5 * m + k] = W2[k, j] / 16.0

    eff = np.minimum(np.arange(1, PCH + 1, dtype=np.float32), np.float32(WIN))
    r0 = (np.float32(1.0) / (eff + np.float32(1e-12))).astype(np.float32)
    r0b = np.broadcast_to(r0[:, None], (PCH, BC)).copy()
    return convm, w1blk, b1vec, w2blk, r0b


def _build_graph(b2, dbg=False, phases=3):
    nc = bacc.Bacc()
    x_d = nc.declare_dram_parameter("x", [BL, T, C], F32, isOutput=False)
    convr_d = nc.declare_dram_parameter("convmr", [PCH, 12, PCH], F32R, isOutput=False)
    w1r_d = nc.declare_dram_parameter("w1blkr", [8, 128], F16, isOutput=False)
    w2r_d = nc.declare_dram_parameter("w2blkr", [128, 6 * 120], F32R, isOutput=False)
    x0r_d = nc.declare_dram_parameter("x0repr", [PCH, BC], F32R, isOutput=False)
    b1_d = nc.declare_dram_parameter("b1vec", [128, 1], F32, isOutput=False)
    r0_d = nc.declare_dram_parameter("r0b", [PCH, BC], F32, isOutput=False)
    x0_d = nc.declare_dram_parameter("x0rep", [PCH, BC], F32, isOutput=False)
    bias_d = nc.declare_dram_parameter("biasv", [PCH, 7], F32, isOutput=False)
    out_d = nc.declare_dram_parameter("out", [BL, T, C], F32, isOutput=True)
    # logits scratch [k, p4, m, bc]
    TP = 4224  # T padded to a multiple of 384 (48-pos units x 128-pos chunks)
    if dbg:
        lg_d = nc.declare_dram_parameter("lgdbg", [TP // 4, 4, NK, BC], F16, isOutput=True)
        zl_d = nc.declare_dram_parameter("zldbg", [TP // 4, 4, 2, BC], F16, isOutput=True)
    else:
        lg_d = nc.dram_tensor("lgscratch", [TP // 4, 4, NK, BC], F16)
        zl_d = nc.dram_tensor("zlscratch", [TP // 4, 4, 2, BC], F16)

    inv_t = float(16.0 / TEMP)

    with TileContext(nc) as tc:
        with tc.tile_pool(name="const", bufs=1) as cpool:

            b1_sb = cpool.tile([128, 1], F32)
            nc.sync.dma_start(out=b1_sb, in_=b1_d[:])
            r0_sb = cpool.tile([PCH, BC], F32)
            nc.sync.dma_start(out=r0_sb, in_=r0_d[:])
            x0_sb = cpool.tile([PCH, BC], F32)
            nc.sync.dma_start(out=x0_sb, in_=x0_d[:])
            x02_sb = cpool.tile([PCH, BC], F32R)
            bias_sb = cpool.tile([PCH, 7], F32)
            nc.sync.dma_start(out=bias_sb, in_=bias_d[:])
            conv_sbr = cpool.tile([PCH, 12 * PCH], F32R)
            nc.sync.dma_start(out=conv_sbr, in_=convr_d.rearrange("p i q -> p (i q)"))
            w1_sbr = cpool.tile([8, 128], F16)
            nc.sync.dma_start(out=w1_sbr, in_=w1r_d[:])
            w2_sbr = cpool.tile([128, 6 * 120], F32R)
            nc.sync.dma_start(out=w2_sbr, in_=w2r_d[:])
            x0_sbr = cpool.tile([PCH, BC], F32R)
            nc.sync.dma_start(out=x0_sbr, in_=x0r_d[:])


            with tc.tile_pool(name="feat", bufs=1) as fpool:
                y_all = fpool.tile([PCH, NCH * NK * BC], BF16)  # free = (ch, k, bc)

                # ---------------- Phase A ----------------
                with tc.tile_pool(name="xp", bufs=6) as xpool, \
                     tc.tile_pool(name="x2p", bufs=4) as x2pool, \
                     tc.tile_pool(name="cps", bufs=2, space="PSUM") as cpsum, \
                     tc.tile_pool(name="stat", bufs=6) as spool:
                    nc.vector.tensor_mul(x02_sb, x0_sb, x0_sb)
                    xprev_r, x2prev = x0_sbr, x02_sb
                    for ch in range(NCH):
                        xt = xpool.tile([PCH, BC], F32, tag="x")
                        nc.sync.dma_start(
                            out=xt.rearrange("p (b c) -> p b c", b=BL),
                            in_=x_d[:, ch * PCH:(ch + 1) * PCH, :]
                                .rearrange("b p c -> p b c"))
                        xtr = xpool.tile([PCH, BC], F32R, tag="xr")
                        nc.vector.tensor_copy(xtr, xt)
                        x2 = x2pool.tile([PCH, BC], F32R, tag="x2")
                        nc.vector.tensor_mul(x2, xt, xt)
                        psg = cpsum.tile([PCH, 5 * BC], F32, tag="cvg")
                        pst = cpsum.tile([PCH, 2 * BC], F32, tag="cvs")
                        for f in range(5):
                            nc.tensor.matmul(
                                psg[:, f * BC:(f + 1) * BC],
                                lhsT=conv_sbr[:, (2 * f) * PCH:(2 * f + 1) * PCH],
                                rhs=xtr, start=True, stop=False)
                            nc.tensor.matmul(
                                psg[:, f * BC:(f + 1) * BC],
                                lhsT=conv_sbr[:, (2 * f + 1) * PCH:(2 * f + 2) * PCH],
                                rhs=xprev_r, start=False, stop=True)
                        nc.tensor.matmul(pst[:, 0:BC],
                                         lhsT=conv_sbr[:, 10 * PCH:11 * PCH],
                                         rhs=xtr, start=True, stop=False)
                        nc.tensor.matmul(pst[:, 0:BC],
                                         lhsT=conv_sbr[:, 11 * PCH:12 * PCH],
                                         rhs=xprev_r, start=False, stop=True)
                        nc.tensor.matmul(pst[:, BC:2 * BC],
                                         lhsT=conv_sbr[:, 10 * PCH:11 * PCH],
                                         rhs=x2, start=True, stop=False)
                        nc.tensor.matmul(pst[:, BC:2 * BC],
                                         lhsT=conv_sbr[:, 11 * PCH:12 * PCH],
                                         rhs=x2prev, start=False, stop=True)
                        nc.scalar.activation(
                            out=y_all[:, ch * NK * BC:ch * NK * BC + 4 * BC],
                            in_=psg[:, 0:4 * BC], func=AF.Copy)
                        nc.vector.tensor_copy(
                            y_all[:, ch * NK * BC + 4 * BC:(ch + 1) * NK * BC],
                            psg[:, 4 * BC:5 * BC])
                        msq = spool.tile([PCH, BC], F32, tag="msq")
                        var = spool.tile([PCH, BC], F32, tag="var")
                        if ch == 0:
                            mean = spool.tile([PCH, BC], F32, tag="mean")
                            mean2 = spool.tile([PCH, BC], F32, tag="mean2")
                            nc.vector.tensor_mul(mean, pst[:, 0:BC], r0_sb)
                            nc.vector.tensor_mul(mean2, pst[:, BC:2 * BC], r0_sb)
                            nc.scalar.activation(out=msq, in_=mean, func=AF.Square)
                            nc.vector.tensor_sub(var, mean2, msq)
                        else:
                            nc.scalar.activation(out=msq, in_=pst[:, 0:BC],
                                                 func=AF.Square, scale=1.0 / WIN)
                            nc.vector.scalar_tensor_tensor(
                                var, in0=pst[:, BC:2 * BC], scalar=1.0 / WIN,
                                in1=msq, op0=mybir.AluOpType.mult,
                                op1=mybir.AluOpType.subtract)
                        nc.vector.tensor_scalar_max(var, var, 0.0)
                        lv = spool.tile([PCH, BC], F16, tag="lv")
                        nc.scalar.activation(
                            out=lv, in_=var, func=AF.Ln, bias=bias_sb[:, 0:1])
                        rsq = spool.tile([PCH, BC], F32, tag="rsq")
                        nc.scalar.activation(out=rsq, in_=lv, func=AF.Exp, scale=-0.5,
                                             bias=bias_sb[:, 6:7])
                        xm = spool.tile([PCH, BC], F32, tag="xm")
                        if ch == 0:
                            nc.vector.tensor_sub(xm, xt, mean)
                        else:
                            nc.vector.scalar_tensor_tensor(
                                xm, in0=pst[:, 0:BC], scalar=-1.0 / WIN,
                                in1=xt, op0=mybir.AluOpType.mult,
                                op1=mybir.AluOpType.add)
                        zt = spool.tile([PCH, BC], F16, tag="zt")
                        nc.vector.tensor_mul(zt, xm, rsq)
                        nc.gpsimd.dma_start(
                            out=zl_d[ch * 32:(ch + 1) * 32, :, 0, :]
                                .rearrange("p4 m bc -> (p4 m) bc"),
                            in_=zt)
                        nc.gpsimd.dma_start(
                            out=zl_d[ch * 32:(ch + 1) * 32, :, 1, :]
                                .rearrange("p4 m bc -> (p4 m) bc"),
                            in_=lv)
                        if ch == NCH - 1:
                            nc.gpsimd.dma_start(
                                out=zl_d[1024:1056, :, 0, :]
                                    .rearrange("p4 m bc -> (p4 m) bc"),
                                in_=zt)
                            nc.gpsimd.dma_start(
                                out=zl_d[1024:1056, :, 1, :]
                                    .rearrange("p4 m bc -> (p4 m) bc"),
                                in_=lv)
                        xprev_r, x2prev = xtr, x2

                # ---------------- Phase B ----------------
                if phases < 2:
                    phases_skip = True
                else:
                    phases_skip = False
                with tc.tile_pool(name="x1p", bufs=3) as x1pool, \
                     tc.tile_pool(name="hps", bufs=2, space="PSUM") as hpsum, \
                     tc.tile_pool(name="hsb", bufs=3) as hpool, \
                     tc.tile_pool(name="l2ps", bufs=2, space="PSUM") as l2psum, \
                     tc.tile_pool(name="lcp", bufs=4) as lcpool:
                    for up in range(88 if not phases_skip else 0):  # unit pairs: 48 positions each
                        x1 = x1pool.tile([8, 12 * BC], F16, tag="x1")
                        nc.sync.dma_start(
                            out=x1.rearrange("mf (p4 bc) -> mf p4 bc", p4=12),
                            in_=zl_d[12 * up:12 * up + 12]
                                .rearrange("p4 m f bc -> (m f) p4 bc"))
                        l2 = l2psum.tile([120, 512], F32, tag="l2")
                        for ul in range(2):
                            hp = hpsum.tile([128, 6 * BC], F32, tag="h")
                            for a in range(6):
                                blk = ul * 6 + a
                                nc.tensor.matmul(
                                    hp[:, a * BC:(a + 1) * BC],
                                    lhsT=w1_sbr,
                                    rhs=x1[:, blk * BC:(blk + 1) * BC],
                                    start=True, stop=True)
                            hs = hpool.tile([128, 6 * BC], F32R, tag="hs")
                            nc.scalar.activation(out=hs, in_=hp, func=AF.Gelu, bias=b1_sb)
                            for a in range(6):
                                nc.tensor.matmul(
                                    l2[:, ul * BC:(ul + 1) * BC],
                                    lhsT=w2_sbr[:, a * 120:(a + 1) * 120],
                                    rhs=hs[:, a * BC:(a + 1) * BC],
                                    start=(a == 0), stop=(a == 5))
                        lcp = lcpool.tile([120, 512], F16, tag="lc")
                        nc.vector.tensor_copy(lcp, l2)
                        for ul in range(2):
                            p4a = 12 * up + 6 * ul
                            nc.gpsimd.dma_start(
                                out=lg_d[p4a:p4a + 6, :, :, :]
                                    .rearrange("p4 m k bc -> (p4 m k) bc"),
                                in_=lcp[:, ul * BC:(ul + 1) * BC])

                # ---------------- Phase C/D ----------------
                tc.no_sync_barrier()
                NSC = 8 if phases >= 3 else 0
                SCW = 4 * BC  # 1024
                with tc.tile_pool(name="lgp", bufs=2) as lpool, \
                     tc.tile_pool(name="ep", bufs=2) as epool, \
                     tc.tile_pool(name="dp", bufs=3) as dpool, \
                     tc.tile_pool(name="op", bufs=3) as opool:
                    for sc in range(NSC):
                        et = epool.tile([PCH, NK * SCW], BF16, tag="e")
                        lgt = lpool.tile([PCH, NK * SCW], F16, tag="lg")
                        for k in range(NK):
                            nc.gpsimd.dma_start(
                                out=lgt[:, k * SCW:(k + 1) * SCW]
                                    .rearrange("p (q bc) -> p q bc", q=4),
                                in_=lg_d[sc * 128:(sc + 1) * 128, :, k, :]
                                    .rearrange("(q p4) m bc -> (p4 m) q bc", q=4))
                        if sc == 0:
                            # only t<16 can overflow exp: per-element max-subtract here
                            mx = lpool.tile([PCH, SCW], F16, tag="mx")
                            nc.vector.tensor_max(mx, lgt[:, 0:SCW], lgt[:, SCW:2 * SCW])
                            for k in (2, 3, 4):
                                nc.vector.tensor_max(mx, mx, lgt[:, k * SCW:(k + 1) * SCW])
                            for k in range(NK):
                                dk = lpool.tile([PCH, SCW], F16, tag="dk")
                                nc.vector.tensor_sub(dk, lgt[:, k * SCW:(k + 1) * SCW], mx)
                                nc.scalar.activation(
                                    out=et[:, k * SCW:(k + 1) * SCW], in_=dk,
                                    func=AF.Exp, scale=inv_t, bias=bias_sb[:, 1 + k:2 + k])
                        else:
                            for k in range(NK):
                                nc.scalar.activation(
                                    out=et[:, k * SCW:(k + 1) * SCW],
                                    in_=lgt[:, k * SCW:(k + 1) * SCW],
                                    func=AF.Exp, scale=inv_t, bias=bias_sb[:, 1 + k:2 + k])
                        den = dpool.tile([PCH, SCW], BF16, tag="den")
                        nc.vector.tensor_add(den, et[:, 0:SCW], et[:, SCW:2 * SCW])
                        for k in (2, 3, 4):
                            nc.vector.tensor_add(den, den, et[:, k * SCW:(k + 1) * SCW])
                        ld = dpool.tile([PCH, SCW], F32, tag="ld")
                        nc.scalar.activation(out=ld, in_=den, func=AF.Ln)
                        rd = dpool.tile([PCH, SCW], BF16, tag="rd")
                        nc.scalar.activation(out=rd, in_=ld, func=AF.Exp, scale=-1.0)
                        yv = y_all.rearrange("p (ch k bc) -> p ch k bc", ch=NCH, k=NK)
                        ysl = lambda k: yv[:, 4 * sc:4 * sc + 4, k, :]
                        num = dpool.tile([PCH, SCW], BF16, tag="num")
                        tmp = dpool.tile([PCH, SCW], BF16, tag="tmp")
                        nc.vector.tensor_mul(num, et[:, 0:SCW], ysl(0))
                        for k in range(1, NK):
                            nc.vector.tensor_mul(tmp, et[:, k * SCW:(k + 1) * SCW], ysl(k))
                            nc.vector.tensor_add(num, num, tmp)
                        ot = opool.tile([PCH, SCW], F32, tag="ot")
                        nc.gpsimd.tensor_mul(ot, num, rd)
                        for q in range(4):
                            nc.sync.dma_start(
                                out=out_d[:, sc * 512 + q * 128:sc * 512 + (q + 1) * 128, :]
                                    .rearrange("b p c -> p b c"),
                                in_=ot[:, q * BC:(q + 1) * BC]
                                    .rearrange("p (b c) -> p b c", b=BL))
    nc.compile()
    return nc


def kernel(x, W1, b1, W2, b2, _trace=False, _dbg=False):
    global LAST_EXEC_NS, LAST_RESULTS
    x = np.ascontiguousarray(np.asarray(x, np.float32))
    W1 = np.asarray(W1, np.float32)
    b1 = np.asarray(b1, np.float32)
    W2 = np.asarray(W2, np.float32)
    b2 = np.asarray(b2, np.float32)

    convm, w1blk, b1vec, w2blk, r0b = _build_consts(W1, b1, W2)
    convm = np.ascontiguousarray(convm.transpose(1, 0, 2))  # [pi, i, po]
    bvals = np.concatenate([[np.float32(1e-6)],
                            (b2 / np.float32(TEMP)).astype(np.float32),
                            [np.float32(np.log(0.25))]])
    biasv = np.ascontiguousarray(np.broadcast_to(bvals[None, :], (PCH, 7)).astype(np.float32))
    nc = _build_graph(b2, dbg=_dbg)

    in_maps = []
    for i in range(NCORES):
        xs = x[i * BL:(i + 1) * BL]
        x0rep = np.broadcast_to(
            xs[:, 0, :].reshape(1, BC), (PCH, BC)).copy().astype(np.float32)
        in_maps.append({
            "x": np.ascontiguousarray(xs),
            "convmr": convm,
            "w1blkr": w1blk.astype(np.float16), "b1vec": b1vec,
            "w2blkr": w2blk, "r0b": r0b,
            "x0rep": x0rep, "x0repr": x0rep,
            "biasv": biasv,
        })
    res = run_bass_kernel_spmd(nc, in_maps, list(range(NCORES)), trace=_trace)
    LAST_EXEC_NS = res.exec_time_ns
    LAST_RESULTS = res
    out = np.concatenate([res.results[i]["out"] for i in range(NCORES)], axis=0)
    return out.astype(np.float32)


if __name__ == "__main__":
    rng = np.random.default_rng(0)
    xs = rng.standard_normal((B, T, C)).astype(np.float32)
    W1 = rng.standard_normal((H, 2)).astype(np.float32) * 0.5
    b1 = rng.standard_normal((H,)).astype(np.float32) * 0.1
    W2 = rng.standard_normal((NK, H)).astype(np.float32) * 0.3
    b2 = rng.standard_normal((NK,)).astype(np.float32) * 0.1
    o = kernel(xs, W1, b1, W2, b2)
    print("ran", o.shape, o.dtype)

